# revision 1
# baseline (speedup 1.0000x reference)
"""GCN (3-layer message passing) Trainium2 Bass kernel, 8-way node-sharded.

vs v1: fp16 feature table (pair-packed 256B gather units, parity via
shifted source APs), replicated layer-0 prologue (no first AllGather),
chunk-major table layout enabling chunked (overlappable) AllGathers.
"""

import sys

for _p in ("/opt/trn_rl_repo",):
    if _p not in sys.path:
        sys.path.insert(0, _p)

from dataclasses import dataclass, field

import numpy as np


@dataclass
class Cfg:
    N: int = 50000
    IN: int = 128
    HID: int = 64
    LAYERS: int = 3
    CORES: int = 8
    GB: int = 4        # blocks per processing group
    MAXC: int = 6      # max chunks (x128 idxs) per dma_gather call
    GBUFS: int = 2     # gather-tile buffering
    PACK: int = 2      # fp16 rows per gather unit (2 -> 256B, 4 -> 512B)
    SLIM: bool = True  # gather only the 128B row (256B stride), raw inst
    NEGPAD: bool = False  # pad idx with -1 (descriptor skipped by ucode)
    BAL: bool = False  # degree-balanced node->(core,block) assignment (no win)
    REPL0: bool = True  # replicate layer-0 table compute (skip first AG)
    AGCH: int = 2      # AllGather chunks per layer boundary
    XTCH: int = 8192   # xT_full streaming chunk (columns)

    @property
    def NPC(self):
        return self.N // self.CORES

    @property
    def BPC(self):
        return (self.NPC + 127) // 128

    @property
    def NPAD(self):
        return self.BPC * 128

    @property
    def NTAB(self):
        return self.CORES * self.NPAD

    @property
    def NGROUPS(self):
        return (self.BPC + self.GB - 1) // self.GB

    def group_blocks(self, g):
        return list(range(g * self.GB, min((g + 1) * self.GB, self.BPC)))

    @property
    def chunk_groups(self):
        """AG chunk -> list of groups; contiguous split of groups."""
        ng = self.NGROUPS
        k = max(1, min(self.AGCH, ng))
        out, s = [], 0
        for i in range(k):
            n = (ng - s + (k - i) - 1) // (k - i)
            out.append(list(range(s, s + n)))
            s += n
        return out

    @property
    def chunk_rows(self):
        """AG chunk -> (local_row0, local_row1) padded-space."""
        out = []
        for gs in self.chunk_groups:
            b0 = self.group_blocks(gs[0])[0]
            b1 = self.group_blocks(gs[-1])[-1] + 1
            out.append((b0 * 128, b1 * 128))
        return out


def assign_nodes(edge_index, cfg: Cfg):
    """node -> (core, lrow in padded space); degree-balanced if cfg.BAL."""
    if not cfg.BAL:
        n = np.arange(cfg.N)
        return (n // cfg.NPC).astype(np.int64), (n % cfg.NPC).astype(np.int64)
    import heapq

    row = np.asarray(edge_index[0], dtype=np.int64)
    deg = np.bincount(row, minlength=cfg.N)
    order = np.argsort(-deg, kind="stable")
    NB = cfg.CORES * cfg.BPC
    last_cap = cfg.NPC - 128 * (cfg.BPC - 1)
    cap = np.array(
        [last_cap if b % cfg.BPC == cfg.BPC - 1 else 128 for b in range(NB)],
        dtype=np.int64,
    )
    cnt = np.zeros(NB, dtype=np.int64)
    core_n = np.zeros(cfg.N, dtype=np.int64)
    lrow_n = np.zeros(cfg.N, dtype=np.int64)
    heap = [(0, int(b)) for b in range(NB)]
    heapq.heapify(heap)
    for n in order:
        while True:
            load, b = heapq.heappop(heap)
            if cnt[b] < cap[b]:
                break
        core_n[n] = b // cfg.BPC
        lrow_n[n] = (b % cfg.BPC) * 128 + cnt[b]
        cnt[b] += 1
        if cnt[b] < cap[b]:
            heapq.heappush(heap, (load + int(deg[n]), b))
    return core_n, lrow_n


def renumber(cfg: Cfg, core: np.ndarray, lrow: np.ndarray) -> np.ndarray:
    """(core, padded lrow) -> chunk-major padded table row."""
    rows = np.asarray(cfg.chunk_rows)
    starts = rows[:, 0]
    sizes = rows[:, 1] - rows[:, 0]
    k = np.searchsorted(rows[:, 1], lrow, side="right")
    base = np.concatenate([[0], np.cumsum(sizes * cfg.CORES)])[:-1]
    return base[k] + core * sizes[k] + (lrow - starts[k])


@dataclass
class Sched:
    slots: np.ndarray  # [BPC, PACK] int
    slot_block: list = field(default_factory=list)
    slot_cls: list = field(default_factory=list)
    group_call_slots: list = field(default_factory=list)  # [g][cls] -> (s0,s1)
    block_slot_ranges: list = field(default_factory=list)  # [b][cls] -> (s0,s1)
    total: int = 0

    def finalize(self, cfg: Cfg):
        self.slot_block, self.slot_cls = [], []
        self.group_call_slots = []
        self.block_slot_ranges = [
            [None] * cfg.PACK for _ in range(cfg.BPC)
        ]
        s = 0
        for g in range(cfg.NGROUPS):
            calls = []
            for p in range(cfg.PACK):
                s0 = s
                for b in cfg.group_blocks(g):
                    bs0 = s
                    for _ in range(int(self.slots[b, p])):
                        self.slot_block.append(b)
                        self.slot_cls.append(p)
                        s += 1
                    self.block_slot_ranges[b][p] = (bs0, s)
                calls.append((s0, s))
            self.group_call_slots.append(calls)
        self.total = s


def make_schedule(edge_index: np.ndarray, cfg: Cfg, asg):
    row = np.asarray(edge_index[0], dtype=np.int64)
    col = np.asarray(edge_index[1], dtype=np.int64)
    P = cfg.PACK
    core_n, lrow_n = asg

    core = core_n[row]
    rloc = lrow_n[row]
    blk = rloc // 128
    rrel = (rloc % 128).astype(np.float32)
    cpad = renumber(cfg, core_n[col], lrow_n[col])
    cls = cpad % P
    cidx = (cpad // P).astype(np.int64)
    assert cidx.max() < 32768

    key = (core * cfg.BPC + blk) * P + cls
    order = np.lexsort((cidx, key))
    key_s = key[order]
    rrel_s = rrel[order]
    cidx_s = cidx[order].astype(np.int16)

    bounds = np.searchsorted(
        key_s, np.arange(cfg.CORES * cfg.BPC * P + 1), side="left"
    )
    counts = np.diff(bounds).reshape(cfg.CORES, cfg.BPC, P)

    slots = np.max((counts + 127) // 128, axis=0)  # [BPC, P]
    for b in range(cfg.BPC):
        if slots[b].sum() == 0:
            slots[b, 0] = 1

    sched = Sched(slots=slots)
    sched.finalize(cfg)

    TC = sched.total
    rng = np.random.default_rng(12345)
    per_core = []
    for c in range(cfg.CORES):
        idx_flat = np.zeros((TC, 128), dtype=np.int16)
        rr_flat = np.full((TC, 128), -100.0, dtype=np.float32)
        for b in range(cfg.BPC):
            for p in range(P):
                k = (c * cfg.BPC + b) * P + p
                e0, e1 = bounds[k], bounds[k + 1]
                n = e1 - e0
                s0, s1 = sched.block_slot_ranges[b][p]
                cap = (s1 - s0) * 128
                assert n <= cap, (c, b, p, n, cap)
                # pad with ascending indices continuing the segment's sorted
                # run: same-address padding is an HBM hot-spot (~5x slower),
                # random padding loses stream locality. NEGPAD uses -1
                # (ucode skips the descriptor — but wedges mid-list).
                if cfg.NEGPAD:
                    ci = np.full(cap, -1, dtype=np.int16)
                else:
                    base = int(cidx_s[e1 - 1]) + 1 if n > 0 else (b * 53 + p * 29) % 1000
                    ci = (
                        (base + np.arange(cap, dtype=np.int64))
                        % (cfg.NTAB // P)
                    ).astype(np.int16)
                rv = np.full(cap, -100.0, dtype=np.float32)
                ci[:n] = cidx_s[e0:e1]
                rv[:n] = rrel_s[e0:e1]
                idx_flat[s0:s1] = ci.reshape(-1, 128)
                rr_flat[s0:s1] = rv.reshape(-1, 128)

        idx_tile = np.zeros((128, TC * 8), dtype=np.int16)
        base = idx_flat.reshape(TC, 8, 16).transpose(2, 0, 1).reshape(16, TC * 8)
        for rep in range(8):
            idx_tile[rep * 16:(rep + 1) * 16] = base
        rr_tile = rr_flat.T.astype(np.float16).copy()
        per_core.append((idx_tile, rr_tile))

    return sched, per_core


def host_inputs(x, edge_index, W, b, cfg: Cfg):
    x = np.asarray(x, dtype=np.float32)
    W = np.asarray(W, dtype=np.float32)
    b = np.asarray(b, dtype=np.float32)

    asg = assign_nodes(edge_index, cfg)
    core_n, lrow_n = asg
    sched, per_core = make_schedule(edge_index, cfg, asg)

    row = np.asarray(edge_index[0], dtype=np.int64)
    deg = np.bincount(row, minlength=cfg.N).astype(np.float32)
    norm = 1.0 / np.sqrt(1.0 + deg)

    iota = np.tile(np.arange(128, dtype=np.float16), (128, 1))
    brep = np.tile(b[None, :], (128, 1)).astype(np.float32)

    # renumbered full-table x^T and norm (same for all cores)
    perm = renumber(cfg, core_n, lrow_n)  # node -> table row
    xT_full = np.zeros((cfg.IN, cfg.NTAB), dtype=np.float16)
    xT_full[:, perm] = x.T.astype(np.float16)
    nfull = np.ones(cfg.NTAB, dtype=np.float32)
    nfull[perm] = norm
    norm_full = nfull.reshape(cfg.NTAB // 128, 128).T.copy()  # [128, 392]

    in_maps = []
    for c in range(cfg.CORES):
        sel = core_n == c
        xT = np.zeros((cfg.IN, cfg.NPAD), dtype=np.float16)
        xT[:, lrow_n[sel]] = x[sel].T.astype(np.float16)
        nc_ = np.ones(cfg.NPAD, dtype=np.float32)
        nc_[lrow_n[sel]] = norm[sel]
        normc = nc_.reshape(cfg.BPC, 128).T.copy()
        norm2c = (normc * normc).copy()
        idx_tile, rr_tile = per_core[c]
        m = {
            "xT": xT,
            "Wm": W.astype(np.float16),
            "brep": brep.copy(),
            "iota": iota.copy(),
            "normc": normc,
            "norm2c": norm2c,
            "idx_all": idx_tile,
            "rr_all": rr_tile,
        }
        if cfg.REPL0:
            m["xT_full"] = xT_full.copy()
            m["norm_full"] = norm_full.copy()
        in_maps.append(m)
    return sched, in_maps, asg


def build_bass(cfg: Cfg, sched: Sched, no_ag: bool = False, ablate=()):
    from concourse import bacc, bass, mybir, tile

    f32 = mybir.dt.float32
    f16 = mybir.dt.float16
    i16 = mybir.dt.int16
    EQ = mybir.AluOpType.is_equal
    ADD = mybir.AluOpType.add

    P = cfg.PACK
    TC = sched.total
    nc = bacc.Bacc(
        "TRN2",
        target_bir_lowering=False,
        debug=False,
        num_devices=cfg.CORES,
        num_swdge_queues=4,
    )

    xT_d = nc.dram_tensor("xT", [cfg.IN, cfg.NPAD], f16, kind="ExternalInput")
    W_d = nc.dram_tensor("Wm", [cfg.IN, cfg.HID], f16, kind="ExternalInput")
    brep_d = nc.dram_tensor("brep", [128, cfg.HID], f32, kind="ExternalInput")
    iota_d = nc.dram_tensor("iota", [128, 128], f16, kind="ExternalInput")
    normc_d = nc.dram_tensor("normc", [128, cfg.BPC], f32, kind="ExternalInput")
    norm2c_d = nc.dram_tensor("norm2c", [128, cfg.BPC], f32, kind="ExternalInput")
    idx_d = nc.dram_tensor("idx_all", [128, TC * 8], i16, kind="ExternalInput")
    rr_d = nc.dram_tensor("rr_all", [128, TC], f16, kind="ExternalInput")
    out_d = nc.dram_tensor("out", [cfg.NPAD, cfg.HID], f32, kind="ExternalOutput")
    if cfg.REPL0:
        xTf_d = nc.dram_tensor(
            "xT_full", [cfg.IN, cfg.NTAB], f16, kind="ExternalInput"
        )
        nf_d = nc.dram_tensor(
            "norm_full", [128, cfg.NTAB // 128], f32, kind="ExternalInput"
        )

    xs_in = [
        nc.dram_tensor(f"xs_in{l}", [cfg.NPAD, cfg.HID], f16)
        for l in range(1, cfg.LAYERS)
    ]  # layer l>=1 AG inputs
    _aspace = "Shared" if cfg.CORES > 4 else "Local"
    # flat fp16 table + PACK rows of pad for shifted-pair reads
    xs_full = [
        nc.dram_tensor(
            f"xs_full{l}", [(cfg.NTAB + P) * cfg.HID], f16, addr_space=_aspace
        )
        for l in range(cfg.LAYERS)
    ]

    rg = [list(range(cfg.CORES))]
    UNIT = P * cfg.HID  # fp16 elems per gather unit (256B/512B)
    GE = cfg.HID if cfg.SLIM else UNIT  # gathered elems per edge

    def gather_srcs(l):
        """class p -> per-edge source AP shifted by p rows."""
        srcs = []
        for p in range(P):
            off = p * cfg.HID
            n = cfg.NTAB // P
            ap = xs_full[l][off : off + n * UNIT].rearrange(
                "(n k) -> n k", k=UNIT
            )
            if cfg.SLIM:
                ap = ap[:, : cfg.HID]
            srcs.append(ap)
        return srcs

    def raw_gather(out_ap, in_ap, idxs_ap, num_idxs, queue_num):
        """dma_gather with elem_size < stride (sub-256B payload)."""
        eng = nc.gpsimd
        assert in_ap.ap[0][0] == UNIT, in_ap.ap
        stride_bytes = UNIT * mybir.dt.size(in_ap.dtype)
        sb256, rem = divmod(stride_bytes, 256)
        assert rem == 0 and sb256 < 256
        _in_ap = eng.lower_ap_dma(in_ap, for_custom_bir_dma=True)
        _idxs_ap = eng.lower_ap(idxs_ap)
        _out_ap = eng.lower_ap(out_ap)
        return eng.add_instruction(
            mybir.InstDMAGatherAnt(
                name=eng.bass.get_next_instruction_name(),
                ins=[
                    *_in_ap,
                    _idxs_ap,
                    eng.lower_val_access(eng.to_reg(num_idxs)),
                ],
                outs=[_out_ap],
                transpose=False,
                num_idxs=num_idxs,
                elem_size=GE,
                stride_bytes_256=sb256,
                gen_mode=0,
                single_packet=False,
                queue_num=queue_num,
                sbuf_tokens_per_rank=0,
                sbuf_free_dim_per_rank=0,
                sbuf_free_dim_pad_per_rank=0,
                sbuf_byte_offset=0,
            )
        )

    def table_rows(l, r0, r1):
        """table row range as [rows, HID] fp16 AP."""
        return xs_full[l][r0 * cfg.HID : r1 * cfg.HID].rearrange(
            "(n h) -> n h", h=cfg.HID
        )

    with tile.TileContext(nc) as tc:
        with (
            tc.tile_pool(name="const", bufs=1) as constp,
            tc.tile_pool(name="gbuf", bufs=cfg.GBUFS) as gpool,
            tc.tile_pool(name="onehot", bufs=2) as opool,
            tc.tile_pool(name="xsg", bufs=2) as xsgp,
            tc.tile_pool(name="psum", bufs=8, space="PSUM") as psp,
        ):
            W_s = constp.tile([cfg.IN, cfg.HID], f16, tag="W")
            brep_s = constp.tile([128, cfg.HID], f32, tag="brep")
            iota_s = constp.tile([128, 128], f16, tag="iota")
            normc_s = constp.tile([128, cfg.BPC], f32, tag="normc")
            norm2c_s = constp.tile([128, cfg.BPC], f32, tag="norm2c")
            idx_s = constp.tile([128, TC * 8], i16, tag="idx")
            rr_s = constp.tile([128, TC], f16, tag="rr")
            xs_ping = constp.tile([128, cfg.BPC, cfg.HID], f16, tag="xsA")
            xs_pong = constp.tile([128, cfg.BPC, cfg.HID], f16, tag="xsB")

            nc.sync.dma_start(W_s[:], W_d[:, :])
            nc.sync.dma_start(brep_s[:], brep_d[:, :])
            nc.sync.dma_start(iota_s[:], iota_d[:, :])
            nc.sync.dma_start(normc_s[:], normc_d[:, :])
            nc.sync.dma_start(norm2c_s[:], norm2c_d[:, :])
            nc.sync.dma_start(idx_s[:], idx_d[:, :])
            nc.sync.dma_start(rr_s[:], rr_d[:, :])

            def store_group_rows(dram_ap_fn, g, src_ap):
                blocks = cfg.group_blocks(g)
                b0, b1 = blocks[0], blocks[-1] + 1
                dst = dram_ap_fn(128 * b0, 128 * b1).rearrange(
                    "(b p) h -> p b h", p=128
                )
                nc.sync.dma_start(dst, src_ap)

            # ---- prologue ----
            with tc.tile_pool(name="xtp", bufs=2) as xtp:
                if cfg.REPL0:
                    nfull_s = constp.tile(
                        [128, cfg.NTAB // 128], f32, tag="nfull"
                    )
                    nc.sync.dma_start(nfull_s[:], nf_d[:, :])
                    NB = cfg.NTAB // 128
                    CB = cfg.XTCH // 128
                    for c0 in range(0, NB, CB):
                        c1 = min(c0 + CB, NB)
                        xt_t = xtp.tile([cfg.IN, (c1 - c0) * 128], f16, tag="xt")
                        nc.sync.dma_start(
                            xt_t[:], xTf_d[:, 128 * c0 : 128 * c1]
                        )
                        ot = xtp.tile([128, (c1 - c0), cfg.HID], f16, tag="ot")
                        for j, bb in enumerate(range(c0, c1)):
                            ps = psp.tile([128, cfg.HID], f32, tag="ps")
                            nc.tensor.matmul(
                                ps[:],
                                xt_t[:, 128 * j : 128 * (j + 1)],
                                W_s[:],
                                start=True,
                                stop=True,
                            )
                            tmp = xsgp.tile([128, cfg.HID], f32, tag="t0")
                            nc.vector.tensor_tensor(
                                tmp[:], ps[:], brep_s[:], ADD
                            )
                            nc.vector.tensor_scalar_mul(
                                ot[:, j, :], tmp[:], nfull_s[:, bb : bb + 1]
                            )
                        dst = table_rows(0, 128 * c0, 128 * c1).rearrange(
                            "(b p) h -> p b h", p=128
                        )
                        nc.sync.dma_start(dst, ot[:])
                # own rows -> xs_ping (scaled h0)
                xT_s = xtp.tile([cfg.IN, cfg.NPAD], f16, tag="xTo")
                nc.sync.dma_start(xT_s[:], xT_d[:, :])
                for b in range(cfg.BPC):
                    ps = psp.tile([128, cfg.HID], f32, tag="ps")
                    nc.tensor.matmul(
                        ps[:],
                        xT_s[:, 128 * b : 128 * (b + 1)],
                        W_s[:],
                        start=True,
                        stop=True,
                    )
                    tmp = xsgp.tile([128, cfg.HID], f32, tag="t0")
                    nc.vector.tensor_tensor(tmp[:], ps[:], brep_s[:], ADD)
                    nc.vector.tensor_scalar_mul(
                        xs_ping[:, b, :], tmp[:], normc_s[:, b : b + 1]
                    )
                assert cfg.REPL0, "only REPL0 path implemented"

            # ---- layers ----
            xs_cur, xs_nxt = xs_ping, xs_pong
            for l in range(cfg.LAYERS):
                last = l == cfg.LAYERS - 1
                srcs = gather_srcs(l)
                ag_done = 0
                for g in range(cfg.NGROUPS):
                    blocks = cfg.group_blocks(g)
                    calls = sched.group_call_slots[g]
                    gt = {}
                    for p in range(P):
                        s0, s1 = calls[p]
                        nch = s1 - s0
                        if nch == 0:
                            continue
                        G = gpool.tile([128, nch, GE], f16, tag=f"G{p}")
                        if "gather" in ablate:
                            nc.vector.memset(G[:], 0.0)
                        oh = opool.tile([128, nch, 128], f16, tag=f"oh{p}")
                        for ci, o0 in enumerate(
                            [] if "gather" in ablate else range(0, nch, cfg.MAXC)
                        ):
                            o1 = min(o0 + cfg.MAXC, nch)
                            n = o1 - o0
                            qn = (ci + 2 * p) % 4
                            if cfg.SLIM:
                                raw_gather(
                                    G[:, o0:o1, :],
                                    srcs[p],
                                    idx_s[:, 8 * (s0 + o0) : 8 * (s0 + o1)],
                                    n * 128,
                                    qn,
                                )
                            else:
                                nc.gpsimd.dma_gather(
                                    G[:, o0:o1, :],
                                    srcs[p],
                                    idx_s[:, 8 * (s0 + o0) : 8 * (s0 + o1)],
                                    n * 128,
                                    n * 128,
                                    UNIT,
                                    single_packet=False,
                                    queue_num=qn,
                                )
                        if "oh" in ablate:
                            nc.vector.memset(oh[:], 0.0)
                        else:
                            rr_b = (
                                rr_s[:, s0:s1]
                                .unsqueeze(2)
                                .broadcast_to([128, nch, 128])
                            )
                            io_b = (
                                iota_s[:, :]
                                .unsqueeze(1)
                                .broadcast_to([128, nch, 128])
                            )
                            nc.vector.tensor_tensor(oh[:], io_b, rr_b, EQ)
                        gt[p] = (G, oh, s0)

                    for b in blocks:
                        ps = psp.tile([128, cfg.HID], f32, tag="ps")
                        mm = []
                        for p in range(P):
                            bs0, bs1 = sched.block_slot_ranges[b][p]
                            if bs1 > bs0:
                                G, oh, s0 = gt[p]
                                for s in range(bs0, bs1):
                                    mm.append((G, oh, s - s0))
                        assert mm, f"block {b} has no slots"
                        if "mm" in ablate:
                            nc.vector.memset(ps[:], 0.0)
                            mm = []
                        for k, (G, oh, sl) in enumerate(mm):
                            nc.tensor.matmul(
                                ps[:],
                                oh[:, sl, :],
                                G[:, sl, : cfg.HID],
                                start=(k == 0),
                                stop=(k == len(mm) - 1),
                            )
                        nsrc = normc_s if last else norm2c_s
                        tmp = xsgp.tile([128, cfg.HID], f32, tag="t1")
                        nc.vector.tensor_tensor(
                            tmp[:], ps[:], xs_cur[:, b, :], ADD
                        )
                        if last:
                            ot = xsgp.tile([128, cfg.HID], f32, tag="t2")
                            nc.vector.tensor_scalar_mul(
                                ot[:], tmp[:], nsrc[:, b : b + 1]
                            )
                            dst = out_d[128 * b : 128 * (b + 1), :].rearrange(
                                "(o p) h -> p o h", p=128
                            )
                            nc.sync.dma_start(dst, ot[:])
                        else:
                            nc.vector.tensor_scalar_mul(
                                xs_nxt[:, b, :], tmp[:], nsrc[:, b : b + 1]
                            )
                    if not last:
                        blocks = cfg.group_blocks(g)
                        dst = xs_in[l][
                            128 * blocks[0] : 128 * (blocks[-1] + 1), :
                        ].rearrange("(o p) h -> p o h", p=128)
                        nc.sync.dma_start(
                            dst, xs_nxt[:, blocks[0] : blocks[-1] + 1, :]
                        )
                        # chunked AG as soon as a chunk's groups are stored
                        if not no_ag:
                            cg = cfg.chunk_groups
                            if ag_done < len(cg) and g == cg[ag_done][-1]:
                                r0, r1 = cfg.chunk_rows[ag_done]
                                t0 = sum(
                                    (rr1 - rr0) * cfg.CORES
                                    for rr0, rr1 in cfg.chunk_rows[:ag_done]
                                )
                                tsz = (r1 - r0) * cfg.CORES
                                nc.gpsimd.collective_compute(
                                    "AllGather",
                                    mybir.AluOpType.bypass,
                                    replica_groups=rg,
                                    ins=[xs_in[l][r0:r1, :]],
                                    outs=[table_rows(l + 1, t0, t0 + tsz)],
                                )
                                ag_done += 1
                if not last:
                    xs_cur, xs_nxt = xs_nxt, xs_cur

    nc.compile()
    return nc


def bench_bass(nc, in_maps, n_cores, iters=20, warmup=2):
    """Repeat-execution device benchmark (no NTFF tracing in this container).

    Mirrors bass2jax.run_bass_via_pjrt's multi-core path, minus output-buffer
    donation so the compiled executable can be re-invoked. Returns
    (results_list, per_iter_seconds).
    """
    import time

    import jax
    from jax.experimental.shard_map import shard_map
    from jax.sharding import Mesh, NamedSharding, PartitionSpec

    from concourse import bass2jax, mybir

    bass2jax.install_neuronx_cc_hook()

    partition_name = (
        nc.partition_id_tensor.name if nc.partition_id_tensor else None
    )
    in_names, out_names, out_avals, zero_outs = [], [], [], []
    for alloc in nc.m.functions[0].allocations:
        if not isinstance(alloc, mybir.MemoryLocationSet):
            continue
        name = alloc.memorylocations[0].name
        if alloc.kind == "ExternalInput":
            if name != partition_name:
                in_names.append(name)
        elif alloc.kind == "ExternalOutput":
            out_names.append(name)
            shape = tuple(alloc.tensor_shape)
            dtype = mybir.dt.np(alloc.dtype)
            out_avals.append(jax.core.ShapedArray(shape, dtype))
            zero_outs.append(np.zeros(shape, dtype))
    n_params = len(in_names)
    all_names = in_names + out_names
    if partition_name is not None:
        all_names = all_names + [partition_name]

    def _body(*args):
        operands = list(args)
        if partition_name is not None:
            operands.append(bass2jax.partition_id_tensor())
        outs = bass2jax._bass_exec_p.bind(
            *operands,
            out_avals=tuple(out_avals),
            in_names=tuple(all_names),
            out_names=tuple(out_names),
            lowering_input_output_aliases=(),
            sim_require_finite=True,
            sim_require_nnan=True,
            nc=nc,
        )
        return tuple(outs)

    devices = jax.devices()[:n_cores]
    mesh = Mesh(np.asarray(devices), ("core",))
    spec = PartitionSpec("core")
    nin = n_params + len(zero_outs)
    sharded = jax.jit(
        shard_map(
            _body,
            mesh=mesh,
            in_specs=(spec,) * nin,
            out_specs=(spec,) * len(out_names),
            check_rep=False,
        ),
        keep_unused=True,
    )
    sh = NamedSharding(mesh, spec)
    args = [
        jax.device_put(
            np.concatenate([np.asarray(m[name]) for m in in_maps], axis=0), sh
        )
        for name in in_names
    ] + [
        jax.device_put(
            np.zeros((n_cores * z.shape[0], *z.shape[1:]), z.dtype), sh
        )
        for z in zero_outs
    ]

    out_arrs = None
    for _ in range(warmup):
        out_arrs = sharded(*args)
        jax.block_until_ready(out_arrs)
    t0 = time.perf_counter()
    for _ in range(iters):
        out_arrs = sharded(*args)
    jax.block_until_ready(out_arrs)
    t1 = time.perf_counter()

    results = [
        {
            name: np.asarray(out_arrs[i]).reshape(n_cores, *out_avals[i].shape)[c]
            for i, name in enumerate(out_names)
        }
        for c in range(n_cores)
    ]
    return results, (t1 - t0) / iters



def unshard(results, asg, cfg: Cfg):
    core_n, lrow_n = asg
    stacked = np.stack([r["out"] for r in results], axis=0)  # [C, NPAD, H]
    return stacked[core_n, lrow_n]


def kernel(x, edge_index, W, b, cfg: Cfg | None = None):
    cfg = cfg or Cfg()
    sched, in_maps, asg = host_inputs(x, edge_index, W, b, cfg)
    nc = build_bass(cfg, sched)

    from concourse import bass_utils

    res = bass_utils.run_bass_kernel_spmd(
        nc, in_maps, core_ids=list(range(cfg.CORES)), trace=False
    )
    return unshard(res.results, asg, cfg)


if __name__ == "__main__":
    pass

